# revision 3
# baseline (speedup 1.0000x reference)
"""GQA attention block (QKV proj + RoPE + causal attention + out proj),
tensor-parallel over 8 TRN2 NeuronCores.

Sharding: q-heads/kv-heads split across cores (4 q-heads + 1 kv-head each),
wq/wk/wv column-split by head, attention computed locally per head, attention
outputs all-gathered (feature-major), wo column-split so each core produces
out[:, c*512:(c+1)*512]; host concatenates.

Compute dtype: fp16 on the PE (fp32 PSUM accumulation). All operands are
pre-transposed on the host so no on-chip transposes are needed except
vT -> v (DMA transpose).

Schedule: both batches' attention run back-to-back with each AllGather issued
as soon as its batch's attention finishes, so AG(b0) overlaps attention(b1)
and AG(b1) overlaps out-proj(b0); xt triple-buffered so phase-1 chunk
boundaries don't stall the PE; rowsum PSUM double-buffered so the [1,512]
reciprocal (3.3us on DVE) stays off the PE critical path.
"""

import sys

sys.path.insert(0, "/opt/trn_rl_repo")

import numpy as np

import concourse.bass as bass
import concourse.mybir as mybir
import concourse.tile as tile
from concourse import bacc
from concourse.bass_utils import run_bass_kernel_spmd

N_CORES = 8
B = 2
DIM = 4096
H = 32
KVH = 8
HD = 128
HL = H // N_CORES          # 4 local q-heads
KC = DIM // 128            # 32 contraction chunks
FP16 = mybir.dt.float16
FP32 = mybir.dt.float32

SKIP, PLAIN = -1, 0


def classify_mask(mask):
    """Per (512-q-chunk, 128-k-tile) block classification of the additive mask.

    Returns (cls, pats): cls[j][kt] in {SKIP, PLAIN, 1+pat_idx};
    pats: list of distinct [128, 512] fp32 additive mask tiles (mask.T blocks),
    added to the scores before exp.
    """
    S = mask.shape[0]
    nch, nkt = S // 512, S // 128
    m = mask.astype(np.float32)
    em = np.exp(m.astype(np.float64))
    cls = [[PLAIN] * nkt for _ in range(nch)]
    pats = []
    keys = {}
    for j in range(nch):
        for kt in range(nkt):
            qs, ks = slice(j * 512, (j + 1) * 512), slice(kt * 128, (kt + 1) * 128)
            if not em[qs, ks].any():
                cls[j][kt] = SKIP
            elif (m[qs, ks] == 0.0).all():
                cls[j][kt] = PLAIN
            else:
                blk = np.ascontiguousarray(m[qs, ks].T)  # [128k, 512q] fp32
                k = blk.tobytes()
                if k not in keys:
                    keys[k] = len(pats)
                    pats.append(blk)
                cls[j][kt] = 1 + keys[k]
    return cls, pats


def build(S, cls, npat, n_iters=1):
    nch = S // 512   # 512-token chunks per batch
    nkt = S // 128   # 128-token k tiles per batch

    nc = bacc.Bacc("TRN2", target_bir_lowering=False, debug=False,
                   num_devices=N_CORES)

    xT = nc.declare_dram_parameter("xT", [B, DIM, S], FP16, isOutput=False)
    wqT = nc.declare_dram_parameter("wqT", [DIM, HL * HD], FP16, isOutput=False)
    wkT = nc.declare_dram_parameter("wkT", [DIM, HD], FP16, isOutput=False)
    wvT = nc.declare_dram_parameter("wvT", [DIM, HD], FP16, isOutput=False)
    woT = nc.declare_dram_parameter("woT", [DIM, 512], FP16, isOutput=False)
    csP = nc.declare_dram_parameter("cs", [128, S], FP32, isOutput=False)
    csqP = nc.declare_dram_parameter("csq", [128, S], FP32, isOutput=False)
    patP = nc.declare_dram_parameter("pats", [128, max(npat, 1) * 512], FP32,
                                     isOutput=False)
    outP = nc.declare_dram_parameter("out", [B, S, 512], FP16, isOutput=True)

    with tile.TileContext(nc) as tc:
        with (
            tc.tile_pool(name="wpool", bufs=1) as wpool,
            tc.tile_pool(name="qkv", bufs=1) as qkv,
            tc.tile_pool(name="dram", bufs=1, space="DRAM") as dram,
        ):
            # resident weights + constants; wq loaded first (first MMs need it)
            wq_sb = wpool.tile([128, KC, HL * HD], FP16)
            wk_sb = wpool.tile([128, KC, HD], FP16)
            wv_sb = wpool.tile([128, KC, HD], FP16)
            pats_sb = wpool.tile([128, max(npat, 1) * 512], FP32)
            ones_sb = wpool.tile([128, 1], FP16)
            ones1_sb = wpool.tile([1, 128], FP32)
            nc.sync.dma_start(out=wq_sb[:], in_=wqT[:, :].rearrange("(c p) n -> p c n", p=128))
            nc.any.memset(ones_sb[:], 1.0)
            nc.any.memset(ones1_sb[:], 1.0)

            # phase-1 outputs (resident through phase 2)
            qT_sb = qkv.tile([128, B, HL, S], FP16)       # [d, b, h, t]
            kT_sb = qkv.tile([128, B, S], FP16)           # [d, b, t]
            v_sb = qkv.tile([128, B, nkt, HD], FP16)      # [t%128, b, kt, d]

            for _it in range(n_iters):
                # ---------------- phase 1: QKV projection + RoPE ----------------
                with (
                    tc.tile_pool(name="cspool", bufs=1) as cspool,
                    tc.tile_pool(name="xt", bufs=3) as xtp,
                    tc.tile_pool(name="ph1ps", bufs=3, space="PSUM") as ph1ps,
                    tc.tile_pool(name="ph1tmp", bufs=2) as ph1tmp,
                ):
                    def load_xt(b, tcn):
                        t0 = tcn * 512
                        xts = []
                        for half in range(2):
                            xt_h = xtp.tile([128, KC // 2, 512], FP16, tag="xt")
                            nc.sync.dma_start(
                                out=xt_h[:],
                                in_=xT[b, half * (DIM // 2):(half + 1) * (DIM // 2),
                                       t0:t0 + 512].rearrange("(c p) t -> p c t", p=128))
                            xts.append(xt_h)
                        return xts

                    # chunk (0,0) x-load goes out right after wq; the rest of
                    # the constants queue behind it (needed later than wq/x).
                    pending = load_xt(0, 0)
                    cs_sb = cspool.tile([128, S], FP32)
                    csq_sb = cspool.tile([128, S], FP32)
                    if _it == 0:
                        nc.sync.dma_start(out=wk_sb[:], in_=wkT[:, :].rearrange("(c p) n -> p c n", p=128))
                        nc.sync.dma_start(out=wv_sb[:], in_=wvT[:, :].rearrange("(c p) n -> p c n", p=128))
                    nc.sync.dma_start(out=csq_sb[:], in_=csqP[:, :])
                    nc.sync.dma_start(out=cs_sb[:], in_=csP[:, :])
                    if _it == 0:
                        nc.sync.dma_start(out=pats_sb[:], in_=patP[:, :])

                    def rope_evict(ps, dst, cst, t0):
                        c = cst[0:64, t0:t0 + 512]
                        s = cst[64:128, t0:t0 + 512]
                        qE = ps[0:64, :]
                        qO = ps[64:128, :]
                        t1 = ph1tmp.tile([64, 512], FP32, tag="t1")
                        t2 = ph1tmp.tile([64, 512], FP32, tag="t2")
                        t3 = ph1tmp.tile([64, 512], FP32, tag="t3")
                        t4 = ph1tmp.tile([64, 512], FP32, tag="t4")
                        nc.vector.tensor_mul(t1[:], qE, c)
                        nc.vector.tensor_mul(t2[:], qO, s)
                        nc.vector.tensor_sub(dst[0:64, :], t1[:], t2[:])
                        nc.vector.tensor_mul(t3[:], qE, s)
                        nc.vector.tensor_mul(t4[:], qO, c)
                        nc.vector.tensor_add(dst[64:128, :], t3[:], t4[:])

                    chunks = [(b, tcn) for b in range(B) for tcn in range(nch)]
                    for ci, (b, tcn) in enumerate(chunks):
                        t0 = tcn * 512
                        xts = pending
                        if ci + 1 < len(chunks):
                            pending = load_xt(*chunks[ci + 1])

                        def proj(w_sb, n0, psum):
                            for c in range(KC):
                                nc.tensor.matmul(
                                    psum[:],
                                    lhsT=w_sb[:, c, n0:n0 + 128],
                                    rhs=xts[c // (KC // 2)][:, c % (KC // 2), :],
                                    start=(c == 0), stop=(c == KC - 1))

                        for h in range(HL):
                            qp = ph1ps.tile([128, 512], FP32, tag="qk")
                            proj(wq_sb, h * HD, qp)
                            rope_evict(qp, qT_sb[:, b, h, t0:t0 + 512], csq_sb, t0)
                        kp = ph1ps.tile([128, 512], FP32, tag="qk")
                        proj(wk_sb, 0, kp)
                        rope_evict(kp, kT_sb[:, b, t0:t0 + 512], cs_sb, t0)
                        vp = ph1ps.tile([128, 512], FP32, tag="qk")
                        proj(wv_sb, 0, vp)
                        vt_tmp = ph1tmp.tile([128, 512], FP16, tag="vt")
                        nc.scalar.copy(vt_tmp[:], vp[:])
                        for tt in range(4):
                            nc.sync.dma_start_transpose(
                                out=v_sb[:, b, tcn * 4 + tt, :],
                                in_=vt_tmp[:, tt * 128:(tt + 1) * 128])

                # ---------------- phase 2 + 3: attention, AG, out proj ----------------
                with (
                    tc.tile_pool(name="wopool", bufs=1) as wopool,
                    tc.tile_pool(name="stps", bufs=2, space="PSUM") as stps,
                    tc.tile_pool(name="atps", bufs=2, space="PSUM") as atps,
                    tc.tile_pool(name="rsps", bufs=2, space="PSUM") as rsps,
                    tc.tile_pool(name="ph3ps", bufs=2, space="PSUM") as ph3ps,
                    tc.tile_pool(name="ptp", bufs=3) as ptp,
                    tc.tile_pool(name="attnp", bufs=3) as attnp,
                    tc.tile_pool(name="gatp", bufs=2) as gatp,
                    tc.tile_pool(name="outp", bufs=2) as outp,
                    tc.tile_pool(name="smallp", bufs=2) as smallp,
                ):
                    wo_sb = wopool.tile([128, KC, 512], FP16)
                    nc.sync.dma_start(out=wo_sb[:], in_=woT[:, :].rearrange("(c p) n -> p c n", p=128))

                    nloc = nch * HL
                    agins, agouts = [], []
                    for b in range(B):
                        agins.append(dram.tile([nloc, 128, 512], FP16,
                                                name=f"agin{b}", tag=f"agin{b}"))
                        agouts.append(dram.tile([N_CORES * nloc, 128, 512], FP16,
                                                addr_space="Shared",
                                                name=f"agout{b}", tag=f"agout{b}"))

                    # phase 2 for both batches; AG(b) issued as soon as batch b's
                    # attention is done so it overlaps the next compute block.
                    for b in range(B):
                        agin = agins[b]
                        for qc in range(nch):
                            q0 = qc * 512
                            ktlist = [kt for kt in range(nkt) if cls[qc][kt] != SKIP]
                            first, last = ktlist[0], ktlist[-1]
                            for h in range(HL):
                                at = atps.tile([128, 512], FP32, tag="at")
                                rs = rsps.tile([1, 512], FP32, tag="rs")
                                for kt in ktlist:
                                    st = stps.tile([128, 512], FP32, tag="st")
                                    nc.tensor.matmul(
                                        st[:],
                                        lhsT=kT_sb[:, b, kt * 128:(kt + 1) * 128],
                                        rhs=qT_sb[:, b, h, q0:q0 + 512],
                                        start=True, stop=True)
                                    pid = cls[qc][kt]
                                    if pid > 0:
                                        nc.vector.tensor_add(
                                            st[:], st[:],
                                            pats_sb[:, (pid - 1) * 512:pid * 512])
                                    pt = ptp.tile([128, 512], FP16, tag="pt")
                                    nc.scalar.activation(pt[:], st[:],
                                                         mybir.ActivationFunctionType.Exp)
                                    nc.tensor.matmul(at[:], lhsT=v_sb[:, b, kt, :],
                                                     rhs=pt[:],
                                                     start=(kt == first), stop=(kt == last))
                                    nc.tensor.matmul(rs[0:1, :], lhsT=ones_sb[:],
                                                     rhs=pt[:],
                                                     start=(kt == first), stop=(kt == last))
                                # normalization: recip(rowsum) broadcast via outer product
                                rcp = smallp.tile([1, 512], FP32, tag="rcp")
                                nc.vector.reciprocal(rcp[:], rs[0:1, :])
                                bc = stps.tile([128, 512], FP32, tag="st")
                                nc.tensor.matmul(bc[:], lhsT=ones1_sb[:], rhs=rcp[:],
                                                 start=True, stop=True)
                                bcs = smallp.tile([128, 512], FP32, tag="bcs")
                                nc.scalar.copy(bcs[:], bc[:])
                                attn = attnp.tile([128, 512], FP16, tag="attn")
                                nc.vector.tensor_mul(attn[:], at[:], bcs[:])
                                nc.sync.dma_start(
                                    out=agin[qc * HL + h], in_=attn[:])

                        nc.gpsimd.collective_compute(
                            "AllGather", mybir.AluOpType.bypass,
                            replica_groups=[list(range(N_CORES))],
                            ins=[agin.opt()], outs=[agouts[b].opt()])

                    # phase 3 for both batches (AGs complete while this runs)
                    for b in range(B):
                        agout = agouts[b]
                        for qc in range(nch):
                            q0 = qc * 512
                            # gather chunks (r, h) at local offset qc*HL + h
                            rview = agout.rearrange("(r c) d t -> r c d t", r=N_CORES)
                            base = qc * HL
                            gats = []
                            for half in range(2):
                                g = gatp.tile([128, 4, HL, 512], FP16, tag="gat")
                                for rl in range(4):
                                    nc.sync.dma_start(
                                        out=g[:, rl, :, :],
                                        in_=rview[half * 4 + rl,
                                                  base:base + HL, :, :].rearrange(
                                            "h d t -> d h t"))
                                gats.append(g)
                            for tt in range(4):
                                ps3 = ph3ps.tile([128, 512], FP32, tag="ph3")
                                for c in range(KC):
                                    i = c % 16
                                    nc.tensor.matmul(
                                        ps3[:],
                                        lhsT=gats[c // 16][:, i // HL, i % HL,
                                                           tt * 128:(tt + 1) * 128],
                                        rhs=wo_sb[:, c, :],
                                        start=(c == 0), stop=(c == KC - 1))
                                ot = outp.tile([128, 512], FP16, tag="outsb")
                                nc.scalar.copy(ot[:], ps3[:])
                                nc.sync.dma_start(
                                    out=outP[b, q0 + tt * 128:q0 + (tt + 1) * 128, :],
                                    in_=ot[:])
    nc.compile()
    return nc


def make_inputs(x, wq, wk, wv, wo, freqs_cos, freqs_sin, mask):
    """Host-side sharding/transposes. Returns (in_maps, cls, npat, S)."""
    S = x.shape[1]
    perm = np.concatenate([np.arange(0, HD, 2), np.arange(1, HD, 2)])
    xTn = np.ascontiguousarray(np.asarray(x, dtype=np.float32).transpose(0, 2, 1)).astype(np.float16)
    cs = np.ascontiguousarray(
        np.concatenate([np.asarray(freqs_cos).T, np.asarray(freqs_sin).T], axis=0)
    ).astype(np.float32)
    csq = (cs * (1.0 / np.sqrt(HD))).astype(np.float32)
    cls, pats = classify_mask(np.asarray(mask, dtype=np.float32))
    npat = len(pats)
    if npat:
        patA = np.ascontiguousarray(np.concatenate(list(pats), axis=1)).astype(np.float32)
    else:
        patA = np.zeros((128, 512), np.float32)
    wq_, wk_, wv_, wo_ = (np.asarray(a, dtype=np.float32) for a in (wq, wk, wv, wo))

    in_maps = []
    for c in range(N_CORES):
        qrows = np.concatenate([c * (HL * HD) + h * HD + perm for h in range(HL)])
        krows = c * HD + perm
        in_maps.append({
            "xT": xTn,
            "wqT": np.ascontiguousarray(wq_[qrows].T).astype(np.float16),
            "wkT": np.ascontiguousarray(wk_[krows].T).astype(np.float16),
            "wvT": np.ascontiguousarray(wv_[c * HD:(c + 1) * HD].T).astype(np.float16),
            "woT": np.ascontiguousarray(wo_[c * 512:(c + 1) * 512].T).astype(np.float16),
            "cs": cs,
            "csq": csq,
            "pats": patA,
        })
    return in_maps, cls, npat, S


_build_cache = {}


def kernel(x, wq, wk, wv, wo, freqs_cos, freqs_sin, mask, start_pos=0, **_):
    in_maps, cls, npat, S = make_inputs(x, wq, wk, wv, wo, freqs_cos, freqs_sin, mask)
    key = (S, npat, str(cls))
    if key not in _build_cache:
        _build_cache[key] = build(S, cls, npat)
    nc = _build_cache[key]
    res = run_bass_kernel_spmd(nc, in_maps, core_ids=list(range(N_CORES)), trace=False)
    outs = [res.results[c]["out"].reshape(B, S, 512) for c in range(N_CORES)]
    return np.concatenate(outs, axis=2).astype(np.float32)


# revision 14
# speedup vs baseline: 12.2382x; 12.2382x over previous
"""GQA attention block (QKV proj + RoPE + causal attention + out proj),
tensor-parallel over 8 TRN2 NeuronCores.

Sharding: q-heads/kv-heads split across cores (4 q-heads + 1 kv-head each),
wq/wk/wv column-split by head, attention computed locally per head, attention
outputs all-gathered (feature-major), wo column-split so each core produces
out[:, c*512:(c+1)*512]; host concatenates.

Compute dtype: fp16 on the PE (fp32 PSUM accumulation); attn*V and the
softmax row-sums use fp8e4 DoubleRow matmuls (2 k-tiles per MM) except for
the first 256 keys of each batch's first q-chunk, which stay fp16 (fp8 noise
there isn't softmax-averaged away). Causal mask tiles are added into the
score PSUM by an identity-weight matmul (keeps DVE off the score->exp path),
and each head processes its diagonal (masked) k-tiles first so the head's
tail has no mask dependency. 4 heads' rowsums pack into one PSUM bank at
partitions 0/32/64/96, one reciprocal per q-chunk, and the normalization +
agin DMA for a q-chunk group is deferred behind the next group's matmuls so
the reciprocal never stalls the PE. AllGather(b) is issued as soon as batch
b's attention is done, overlapping attention(b+1) / out-proj(b-1).
"""

import sys

sys.path.insert(0, "/opt/trn_rl_repo")

import numpy as np

import concourse.bass as bass
import concourse.mybir as mybir
import concourse.tile as tile
from concourse import bacc
from concourse.bass_utils import run_bass_kernel_spmd

N_CORES = 8
B = 2
DIM = 4096
H = 32
KVH = 8
HD = 128
HL = H // N_CORES          # 4 local q-heads
KC = DIM // 128            # 32 contraction chunks
FP16 = mybir.dt.float16
FP32 = mybir.dt.float32
FP8 = mybir.dt.float8e4

SKIP, PLAIN = -1, 0


def classify_mask(mask):
    """Per (512-q-chunk, 128-k-tile) block classification of the additive mask.

    Returns (cls, pats): cls[j][kt] in {SKIP, PLAIN, 1+pat_idx};
    pats: list of distinct [128, 512] fp32 additive mask tiles (mask.T blocks),
    added to the scores before exp.
    """
    S = mask.shape[0]
    nch, nkt = S // 512, S // 128
    m = mask.astype(np.float32)
    em = np.exp(m.astype(np.float64))
    cls = [[PLAIN] * nkt for _ in range(nch)]
    pats = []
    keys = {}
    for j in range(nch):
        for kt in range(nkt):
            qs, ks = slice(j * 512, (j + 1) * 512), slice(kt * 128, (kt + 1) * 128)
            if not em[qs, ks].any():
                cls[j][kt] = SKIP
            elif (m[qs, ks] == 0.0).all():
                cls[j][kt] = PLAIN
            else:
                blk = np.ascontiguousarray(m[qs, ks].T)  # [128k, 512q] fp32
                k = blk.tobytes()
                if k not in keys:
                    keys[k] = len(pats)
                    pats.append(blk)
                cls[j][kt] = 1 + keys[k]
    return cls, pats


def build(S, cls, npat, n_iters=1):
    nch = S // 512   # 512-token chunks per batch
    nkt = S // 128   # 128-token k tiles per batch

    nc = bacc.Bacc("TRN2", target_bir_lowering=False, debug=False,
                   num_devices=N_CORES)

    xT = nc.declare_dram_parameter("xT", [B, DIM, S], FP16, isOutput=False)
    wqT = nc.declare_dram_parameter("wqT", [DIM, HL * HD], FP16, isOutput=False)
    wkT = nc.declare_dram_parameter("wkT", [DIM, HD], FP16, isOutput=False)
    wvT = nc.declare_dram_parameter("wvT", [DIM, HD], FP16, isOutput=False)
    woT = nc.declare_dram_parameter("woT", [DIM, 512], FP16, isOutput=False)
    csP = nc.declare_dram_parameter("cs", [128, S], FP32, isOutput=False)
    csqP = nc.declare_dram_parameter("csq", [128, S], FP32, isOutput=False)
    patP = nc.declare_dram_parameter("pats", [128, max(npat, 1) * 512], FP16,
                                     isOutput=False)
    eyeP = nc.declare_dram_parameter("eye", [128, 128], FP16, isOutput=False)
    outP = nc.declare_dram_parameter("out", [B, S, 512], FP16, isOutput=True)

    with tile.TileContext(nc) as tc:
        with (
            tc.tile_pool(name="wpool", bufs=1) as wpool,
            tc.tile_pool(name="qkv", bufs=1) as qkv,
            tc.tile_pool(name="dram", bufs=1, space="DRAM") as dram,
        ):
            # resident weights + constants; wq loaded first (first MMs need it)
            wq_sb = wpool.tile([128, KC, HL * HD], FP16)
            wk_sb = wpool.tile([128, KC, HD], FP16)
            wv_sb = wpool.tile([128, KC, HD], FP16)
            pats_sb = wpool.tile([128, max(npat, 1) * 512], FP16)
            eye_sb = wpool.tile([128, 128], FP16)
            ones_sb = wpool.tile([128, 1], FP16)
            ones8_sb = wpool.tile([128, 2, 16], FP8)
            onesbc_sb = wpool.tile([1, 128], FP32)
            nbias_sb = wpool.tile([128, 1], FP32)
            nc.sync.dma_start(out=wq_sb[:], in_=wqT[:, :].rearrange("(c p) n -> p c n", p=128))
            nc.any.memset(ones_sb[:], 1.0)
            nc.any.memset(ones8_sb[:], 1.0)
            nc.any.memset(onesbc_sb[:], 1.0)
            nc.any.memset(nbias_sb[:], -2.0)

            # phase-1 outputs, per batch (separate tiles so phase-2 readers
            # don't conservatively wait on the other batch's writes)
            qT_b = [qkv.tile([128, HL, S], FP16, tag=f"qT{b}", name=f"qT{b}")
                    for b in range(B)]
            kT_b = [qkv.tile([128, S], FP16, tag=f"kT{b}", name=f"kT{b}")
                    for b in range(B)]
            v_b = [qkv.tile([128, nkt, HD], FP16, tag=f"v{b}", name=f"v{b}")
                   for b in range(B)]
            v8_b = [qkv.tile([128, nkt, HD], FP8, tag=f"v8{b}", name=f"v8{b}")
                    for b in range(B)]

            for _it in range(n_iters):
                # ---------------- phase 1: QKV projection + RoPE ----------------
                with (
                    tc.tile_pool(name="cspool", bufs=1) as cspool,
                    tc.tile_pool(name="xt", bufs=3) as xtp,
                    tc.tile_pool(name="ph1ps", bufs=3, space="PSUM") as ph1ps,
                    tc.tile_pool(name="ph1tmp", bufs=2) as ph1tmp,
                ):
                    def load_xt(b, tcn):
                        t0 = tcn * 512
                        xts = []
                        for half in range(2):
                            xt_h = xtp.tile([128, KC // 2, 512], FP16, tag="xt")
                            nc.sync.dma_start(
                                out=xt_h[:],
                                in_=xT[b, half * (DIM // 2):(half + 1) * (DIM // 2),
                                       t0:t0 + 512].rearrange("(c p) t -> p c t", p=128))
                            xts.append(xt_h)
                        return xts

                    # chunk (0,0) x-load goes out right after wq; the rest of
                    # the constants queue behind it (needed later than wq/x).
                    pending = load_xt(0, 0)
                    cs_sb = cspool.tile([128, S], FP32)
                    csq_sb = cspool.tile([128, S], FP32)
                    if _it == 0:
                        nc.sync.dma_start(out=wk_sb[:], in_=wkT[:, :].rearrange("(c p) n -> p c n", p=128))
                        nc.sync.dma_start(out=wv_sb[:], in_=wvT[:, :].rearrange("(c p) n -> p c n", p=128))
                    nc.sync.dma_start(out=csq_sb[:], in_=csqP[:, :])
                    nc.sync.dma_start(out=cs_sb[:], in_=csP[:, :])
                    if _it == 0:
                        nc.sync.dma_start(out=pats_sb[:], in_=patP[:, :])
                        nc.sync.dma_start(out=eye_sb[:], in_=eyeP[:, :])

                    def rope_evict(ps, dst, cst, t0):
                        c = cst[0:64, t0:t0 + 512]
                        s = cst[64:128, t0:t0 + 512]
                        qE = ps[0:64, :]
                        qO = ps[64:128, :]
                        t1 = ph1tmp.tile([64, 512], FP32, tag="t1")
                        t2 = ph1tmp.tile([64, 512], FP32, tag="t2")
                        t3 = ph1tmp.tile([64, 512], FP32, tag="t3")
                        t4 = ph1tmp.tile([64, 512], FP32, tag="t4")
                        nc.vector.tensor_mul(t1[:], qE, c)
                        nc.vector.tensor_mul(t2[:], qO, s)
                        nc.vector.tensor_sub(dst[0:64, :], t1[:], t2[:])
                        nc.vector.tensor_mul(t3[:], qE, s)
                        nc.vector.tensor_mul(t4[:], qO, c)
                        nc.vector.tensor_add(dst[64:128, :], t3[:], t4[:])

                    chunks = [(b, tcn) for b in range(B) for tcn in range(nch)]
                    for ci, (b, tcn) in enumerate(chunks):
                        t0 = tcn * 512
                        xts = pending
                        if ci + 1 < len(chunks):
                            pending = load_xt(*chunks[ci + 1])

                        def proj(w_sb, n0, psum):
                            for c in range(KC):
                                nc.tensor.matmul(
                                    psum[:],
                                    lhsT=w_sb[:, c, n0:n0 + 128],
                                    rhs=xts[c // (KC // 2)][:, c % (KC // 2), :],
                                    start=(c == 0), stop=(c == KC - 1))

                        for h in range(HL):
                            qp = ph1ps.tile([128, 512], FP32, tag="qk")
                            proj(wq_sb, h * HD, qp)
                            rope_evict(qp, qT_b[b][:, h, t0:t0 + 512], csq_sb, t0)
                        kp = ph1ps.tile([128, 512], FP32, tag="qk")
                        proj(wk_sb, 0, kp)
                        rope_evict(kp, kT_b[b][:, t0:t0 + 512], cs_sb, t0)
                        vp = ph1ps.tile([128, 512], FP32, tag="qk")
                        proj(wv_sb, 0, vp)
                        vt_tmp = ph1tmp.tile([128, 512], FP16, tag="vt")
                        nc.scalar.copy(vt_tmp[:], vp[:])
                        for tt in range(4):
                            nc.sync.dma_start_transpose(
                                out=v_b[b][:, tcn * 4 + tt, :],
                                in_=vt_tmp[:, tt * 128:(tt + 1) * 128])
                        nc.scalar.copy(v8_b[b][:, tcn * 4:(tcn + 1) * 4, :],
                                       v_b[b][:, tcn * 4:(tcn + 1) * 4, :])

                # ------------- phase 2 + 3: attention, AG, out proj -------------
                with tc.tile_pool(name="wopool", bufs=1) as wopool:
                    wo_sb = wopool.tile([128, KC, 512], FP16)
                    nc.sync.dma_start(out=wo_sb[:], in_=woT[:, :].rearrange("(c p) n -> p c n", p=128))

                    nloc = nch * HL
                    agins, agouts = [], []
                    for b in range(B):
                        agins.append(dram.tile([nloc, 128, 512], FP16,
                                                name=f"agin{b}", tag=f"agin{b}"))
                        agouts.append(dram.tile([N_CORES * nloc, 128, 512], FP16,
                                                addr_space="Shared",
                                                name=f"agout{b}", tag=f"agout{b}"))

                    with (
                        tc.tile_pool(name="stps", bufs=2, space="PSUM") as stps,
                        tc.tile_pool(name="atps", bufs=2, space="PSUM") as atps,
                        tc.tile_pool(name="rsps", bufs=2, space="PSUM") as rsps,
                        tc.tile_pool(name="ptp", bufs=3) as ptp,
                        tc.tile_pool(name="atsb", bufs=6) as atsbp,
                        tc.tile_pool(name="attnp", bufs=3) as attnp,
                        tc.tile_pool(name="smallp", bufs=8) as smallp,
                    ):
                        def emit_norm(grp):
                            """Deferred normalization for a finished (b, qc)
                            group: bc = outer(ones, 1/rowsum) on PE, attn =
                            at_sb * bc on DVE (bc read straight from PSUM),
                            then DMA into the AG input buffer."""
                            b, qc, ats, rcps = grp
                            for h in range(HL):
                                bc = atps.tile([128, 512], FP32, tag="at")
                                nc.tensor.matmul(
                                    bc[:], lhsT=onesbc_sb[:],
                                    rhs=rcps[h][0:1, :],
                                    start=True, stop=True)
                                attn = attnp.tile([128, 512], FP16, tag="attn")
                                nc.vector.tensor_mul(attn[:], ats[h][:], bc[:])
                                nc.sync.dma_start(
                                    out=agins[b][qc * HL + h], in_=attn[:])

                        prev = None
                        for b in range(B):
                            for qc in range(nch):
                                if prev is not None:
                                    emit_norm(prev)
                                    if prev[1] == nch - 1:
                                        nc.gpsimd.collective_compute(
                                            "AllGather", mybir.AluOpType.bypass,
                                            replica_groups=[list(range(N_CORES))],
                                            ins=[agins[prev[0]].opt()],
                                            outs=[agouts[prev[0]].opt()])
                                q0 = qc * 512
                                # diagonal (partially masked) k-tiles first so
                                # the head's tail tiles have no mask dependency
                                ktl = ([kt for kt in range(nkt)
                                        if cls[qc][kt] not in (SKIP, PLAIN)] +
                                       [kt for kt in range(nkt) if cls[qc][kt] == PLAIN])
                                pairs = [(ktl[2 * i], ktl[2 * i + 1])
                                         for i in range(len(ktl) // 2)]
                                ats = []
                                rcps = []
                                for h in range(HL):
                                    at = atps.tile([128, 512], FP32, tag="at")
                                    rs = rsps.tile([1, 512], FP32, tag="rs")
                                    npair = len(pairs)
                                    for pi, (ka, kb) in enumerate(pairs):
                                        fp16_pair = (qc == 0 and pi == 0)
                                        stp = stps.tile([128, 2, 512], FP32, tag="st")
                                        for j, kt in enumerate((ka, kb)):
                                            pid = cls[qc][kt]
                                            nc.tensor.matmul(
                                                stp[:, j, :],
                                                lhsT=kT_b[b][:, kt * 128:(kt + 1) * 128],
                                                rhs=qT_b[b][:, h, q0:q0 + 512],
                                                start=True, stop=(pid <= 0))
                                            if pid > 0:
                                                nc.tensor.matmul(
                                                    stp[:, j, :], lhsT=eye_sb[:],
                                                    rhs=pats_sb[:, (pid - 1) * 512:pid * 512],
                                                    start=False, stop=True)
                                        first, last = (pi == 0), (pi == npair - 1)
                                        if fp16_pair:
                                            pt = ptp.tile([128, 2, 512], FP16, tag="pt16")
                                            nc.scalar.activation(
                                                pt[:], stp[:],
                                                mybir.ActivationFunctionType.Exp,
                                                bias=nbias_sb[:])
                                            for j, kt in enumerate((ka, kb)):
                                                nc.tensor.matmul(
                                                    at[:], lhsT=v_b[b][:, kt, :],
                                                    rhs=pt[:, j, :],
                                                    start=(first and j == 0), stop=False)
                                                nc.tensor.matmul(
                                                    rs[0:1, :],
                                                    lhsT=ones_sb[:], rhs=pt[:, j, :],
                                                    start=(first and j == 0), stop=False)
                                        else:
                                            pt8 = ptp.tile([128, 2, 512], FP8, tag="pt8")
                                            nc.scalar.activation(
                                                pt8[:], stp[:],
                                                mybir.ActivationFunctionType.Exp,
                                                bias=nbias_sb[:])
                                            USE_DR = True
                                            if USE_DR:
                                                nc.tensor.matmul(
                                                    at[:], lhsT=v8_b[b][:, ka:ka + 2, :],
                                                    rhs=pt8[:],
                                                    start=first and qc != 0, stop=last,
                                                    perf_mode=mybir.MatmulPerfMode.DoubleRow)
                                                nc.tensor.matmul(
                                                    rs[0:1, :],
                                                    lhsT=ones8_sb[:, :, 0:1], rhs=pt8[:],
                                                    start=first and qc != 0, stop=last,
                                                    perf_mode=mybir.MatmulPerfMode.DoubleRow)
                                            else:
                                                for j, kt in enumerate((ka, kb)):
                                                    nc.tensor.matmul(
                                                        at[:], lhsT=v8_b[b][:, kt, :],
                                                        rhs=pt8[:, j, :],
                                                        start=(first and j == 0 and qc != 0),
                                                        stop=(last and j == 1))
                                                    nc.tensor.matmul(
                                                        rs[0:1, :],
                                                        lhsT=ones8_sb[:, 0, 0:1], rhs=pt8[:, j, :],
                                                        start=(first and j == 0 and qc != 0),
                                                        stop=(last and j == 1))
                                    at_sb = atsbp.tile([128, 512], FP16, tag="atsb")
                                    nc.vector.tensor_copy(at_sb[:], at[:])
                                    ats.append(at_sb)
                                    rcp = smallp.tile([1, 512], FP32, tag="rcp")
                                    nc.vector.reciprocal(rcp[:], rs[0:1, :])
                                    rcps.append(rcp)
                                prev = (b, qc, ats, rcps)
                        emit_norm(prev)
                        nc.gpsimd.collective_compute(
                            "AllGather", mybir.AluOpType.bypass,
                            replica_groups=[list(range(N_CORES))],
                            ins=[agins[B - 1].opt()], outs=[agouts[B - 1].opt()])

                    # phase 3 (AGs complete while earlier chunks compute)
                    with (
                        tc.tile_pool(name="ph3ps", bufs=2, space="PSUM") as ph3ps,
                        tc.tile_pool(name="gatp", bufs=2) as gatp,
                        tc.tile_pool(name="outp", bufs=2) as outp,
                    ):
                        for b in range(B):
                            agout = agouts[b]
                            for qc in range(nch):
                                q0 = qc * 512
                                rview = agout.rearrange("(r c) d t -> r c d t", r=N_CORES)
                                base = qc * HL
                                gats = []
                                for half in range(2):
                                    g = gatp.tile([128, 4, HL, 512], FP16, tag="gat")
                                    for rl in range(4):
                                        nc.sync.dma_start(
                                            out=g[:, rl, :, :],
                                            in_=rview[half * 4 + rl,
                                                      base:base + HL, :, :].rearrange(
                                                "h d t -> d h t"))
                                    gats.append(g)
                                for tt in range(4):
                                    ps3 = ph3ps.tile([128, 512], FP32, tag="ph3")
                                    for c in range(KC):
                                        i = c % 16
                                        nc.tensor.matmul(
                                            ps3[:],
                                            lhsT=gats[c // 16][:, i // HL, i % HL,
                                                               tt * 128:(tt + 1) * 128],
                                            rhs=wo_sb[:, c, :],
                                            start=(c == 0), stop=(c == KC - 1))
                                    ot = outp.tile([128, 512], FP16, tag="outsb")
                                    nc.scalar.copy(ot[:], ps3[:])
                                    nc.sync.dma_start(
                                        out=outP[b, q0 + tt * 128:q0 + (tt + 1) * 128, :],
                                        in_=ot[:])
    nc.compile()
    return nc


def make_inputs(x, wq, wk, wv, wo, freqs_cos, freqs_sin, mask):
    """Host-side sharding/transposes. Returns (in_maps, cls, npat, S)."""
    S = x.shape[1]
    perm = np.concatenate([np.arange(0, HD, 2), np.arange(1, HD, 2)])
    xTn = np.ascontiguousarray(np.asarray(x, dtype=np.float32).transpose(0, 2, 1)).astype(np.float16)
    cs = np.ascontiguousarray(
        np.concatenate([np.asarray(freqs_cos).T, np.asarray(freqs_sin).T], axis=0)
    ).astype(np.float32)
    csq = (cs * (1.0 / np.sqrt(HD))).astype(np.float32)
    cls, pats = classify_mask(np.asarray(mask, dtype=np.float32))
    npat = len(pats)
    if npat:
        patA = np.ascontiguousarray(np.concatenate(list(pats), axis=1))
    else:
        patA = np.zeros((128, 512), np.float32)
    patA = np.maximum(patA, -60000.0).astype(np.float16)
    eye = np.eye(128, dtype=np.float16)
    wq_, wk_, wv_, wo_ = (np.asarray(a, dtype=np.float32) for a in (wq, wk, wv, wo))

    in_maps = []
    for c in range(N_CORES):
        qrows = np.concatenate([c * (HL * HD) + h * HD + perm for h in range(HL)])
        krows = c * HD + perm
        in_maps.append({
            "xT": xTn,
            "wqT": np.ascontiguousarray(wq_[qrows].T).astype(np.float16),
            "wkT": np.ascontiguousarray(wk_[krows].T).astype(np.float16),
            "wvT": np.ascontiguousarray(wv_[c * HD:(c + 1) * HD].T).astype(np.float16),
            "woT": np.ascontiguousarray(wo_[c * 512:(c + 1) * 512].T).astype(np.float16),
            "cs": cs,
            "csq": csq,
            "pats": patA,
            "eye": eye,
        })
    return in_maps, cls, npat, S


_build_cache = {}


def kernel(x, wq, wk, wv, wo, freqs_cos, freqs_sin, mask, start_pos=0, **_):
    in_maps, cls, npat, S = make_inputs(x, wq, wk, wv, wo, freqs_cos, freqs_sin, mask)
    key = (S, npat, str(cls))
    if key not in _build_cache:
        _build_cache[key] = build(S, cls, npat)
    nc = _build_cache[key]
    res = run_bass_kernel_spmd(nc, in_maps, core_ids=list(range(N_CORES)), trace=False)
    outs = [res.results[c]["out"].reshape(B, S, 512) for c in range(N_CORES)]
    return np.concatenate(outs, axis=2).astype(np.float32)
